# revision 4
# baseline (speedup 1.0000x reference)
"""Conv2D 3x3 (NCHW, OIHW, stride 1, pad 1) on 8 Trainium2 NeuronCores.

Problem shape: input (32, 128, 56, 56) fp32, weights (256, 128, 3, 3) fp32,
output (32, 256, 56, 56) fp32.

Strategy:
  - Data-parallel over batch: 4 images per core, weights replicated.
  - Host zero-pads images to 58x58 and re-lays weights as [ci, tap, co]
    so the device kernel is pure shifted matmuls.
  - Per image: for each 8-row output piece (8x56 = 448 pixels) and each
    co-half (128 of 256), accumulate 9 tap matmuls in PSUM:
        psum[co, pix] += W[tap][ci, co].T @ x_pad[ci, shifted pixels]
    contract dim = 128 channels (full partitions), moving free dim = 448.
  - fp16 operands and fp16 output DMA (upcast to fp32 on host); error
    budget is 2e-2 and this lands ~4e-4.
  - Dependencies are tile-granular, so the critical first-matmul deps live
    in their own small tiles: weights taps 0-2 of co-half 0 (wa) and input
    rows 0-9 of image 0 (xh). Image 0's remaining low rows ride the vector
    HWDGE ring in parallel with the sync ring. A short warmup burst keeps
    the PE busy (p-state ramp) while those first DMAs land.
"""

import sys

sys.path.insert(0, "/opt/trn_rl_repo")

import numpy as np

N_CORES = 8
N_FULL = 32
IMGS = N_FULL // N_CORES  # images per core
CIN = 128
COUT = 256
H = W = 56
HP = WP = 58  # padded
PIX = H * W  # 3136
PPIX = HP * WP  # 3364

_CACHE = {}

WARMUP_MM = 4


def _split_sync_waits(nc, mybir, max_waits=1):
    """The walrus build in this container rejects instructions carrying
    more than one semaphore wait; hoist extras onto preceding NOPs on the
    same engine (engine executes them in order, semantics preserved)."""
    ctr = 0
    for f in nc.m.functions:
        for bb in f.blocks:
            new_insts = []
            for ins in bb.instructions:
                si = getattr(ins, "sync_info", None)
                if si is not None and si.on_wait and len(si.on_wait) > max_waits:
                    waits = list(si.on_wait)
                    extra, keep = waits[:-max_waits], waits[-max_waits:]
                    for i in range(0, len(extra), max_waits):
                        ctr += 1
                        nop = mybir.InstNoOp(
                            name=f"{ins.name}_wsplit{ctr}",
                            engine=ins.engine,
                            sync_info=mybir.SyncInfo(
                                on_wait=extra[i : i + max_waits], on_update=[]
                            ),
                            bass_nofuse=True,
                        )
                        new_insts.append(nop)
                    si.on_wait = keep
                new_insts.append(ins)
            bb.instructions[:] = new_insts
    return ctr


# input row tiles (padded-row ranges):
#   xh: rows 0..9    (first piece of each image starts as soon as this lands)
#   xa: rows 8..33   (pieces r0=8,16,24)
#   xb: rows 32..57  (pieces r0=32,40,48)
XH_R0, XH_NR = 0, 10
XA_R0, XA_NR = 8, 26
XB_R0, XB_NR = 32, 26


def _groups(n, h):
    """Row-piece groups for image n, co-half h. Each piece is (r0, nrows);
    output pixels r0*56 .. (r0+nrows)*56, PSUM-accumulated over 9 taps."""
    if n == 0 and h == 0:
        # lead with a single piece gated only on the xh + wa tiles
        return [[(0, 8)], [(8, 8), (16, 8), (24, 8)], [(32, 8), (40, 8), (48, 8)]]
    if n == IMGS - 1 and h == 1:
        # trail with a single piece so the final exposed drain + DMA is one
        # transfer
        return [[(0, 8), (8, 8), (16, 8), (24, 8)], [(32, 8), (40, 8)], [(48, 8)]]
    return [[(0, 8), (8, 8), (16, 8), (24, 8)], [(32, 8), (40, 8), (48, 8)]]


def _build():
    import concourse.bass as bass
    import concourse.mybir as mybir
    import concourse.tile as tile

    f32 = mybir.dt.float32
    f16 = mybir.dt.float16

    nc = bass.Bass()
    x = nc.declare_dram_parameter("x", [IMGS, CIN, PPIX], f16, isOutput=False)
    w = nc.declare_dram_parameter("w", [CIN, 9 * COUT], f16, isOutput=False)
    out = nc.declare_dram_parameter("out", [IMGS, COUT, PIX], f16, isOutput=True)

    x4 = x.rearrange("n p (r c) -> n p r c", c=WP)

    with tile.TileContext(nc) as tc:
        with (
            tc.tile_pool(name="wpool", bufs=1) as wpool,
            tc.tile_pool(name="xhpool", bufs=2) as xhpool,
            tc.tile_pool(name="xapool", bufs=2) as xapool,
            tc.tile_pool(name="xbpool", bufs=2) as xbpool,
            tc.tile_pool(name="opool", bufs=2) as opool,
            tc.tile_pool(name="psum", bufs=8, space="PSUM") as pspool,
        ):
            # PE warmup while the first DMAs are in flight (p-state ramp)
            warm = wpool.tile([128, 256], f16, name="warm")
            nc.vector.memzero(warm[:])
            wps = pspool.tile([128, 256], f32, name="ps")
            for _ in range(WARMUP_MM):
                nc.tensor.matmul(
                    wps[:], lhsT=warm[:, 0:128], rhs=warm[:], start=True, stop=True
                )

            # weights on the scalar HWDGE ring, three separate tiles so the
            # first real matmul's LDWEIGHTS waits only on wa (taps 0-2, h0)
            wa = wpool.tile([CIN, 384], f16)
            wb = wpool.tile([CIN, 768], f16)
            wc = wpool.tile([CIN, 1152], f16)
            nc.scalar.dma_start(out=wa[:], in_=w[:, 0:384])
            nc.scalar.dma_start(out=wb[:], in_=w[:, 384:1152])
            nc.scalar.dma_start(out=wc[:], in_=w[:, 1152:2304])

            def lhsT(h, tap):
                if h == 1:
                    return wc[:, tap * 128 : (tap + 1) * 128]
                if tap < 3:
                    return wa[:, tap * 128 : (tap + 1) * 128]
                return wb[:, (tap - 3) * 128 : (tap - 2) * 128]

            for n in range(IMGS):
                xh = xhpool.tile([CIN, XH_NR * WP], f16)
                xa = xapool.tile([CIN, XA_NR * WP], f16)
                xb = xbpool.tile([CIN, XB_NR * WP], f16)
                xh3 = xh.rearrange("p (r c) -> p r c", c=WP)
                xa3 = xa.rearrange("p (r c) -> p r c", c=WP)
                xb3 = xb.rearrange("p (r c) -> p r c", c=WP)
                nc.sync.dma_start(out=xh[:], in_=x4[n, :, XH_R0 : XH_R0 + XH_NR, :])
                nc.sync.dma_start(out=xa[:], in_=x4[n, :, XA_R0 : XA_R0 + XA_NR, :])
                nc.sync.dma_start(out=xb[:], in_=x4[n, :, XB_R0 : XB_R0 + XB_NR, :])

                def rhs(r0, nrows, dy, dx):
                    if r0 == 0:
                        return xh3[:, dy : dy + nrows, dx : dx + W]
                    if r0 + nrows <= 32:
                        ra = r0 + dy - XA_R0
                        return xa3[:, ra : ra + nrows, dx : dx + W]
                    rb = r0 + dy - XB_R0
                    return xb3[:, rb : rb + nrows, dx : dx + W]

                ot = opool.tile([128, 2 * PIX], f16)
                for h in range(2):
                    for grp in _groups(n, h):
                        pss = {}
                        for piece in grp:
                            pss[piece] = pspool.tile([128, 448], f32, name="ps")
                        for tap in range(9):
                            dy, dx = divmod(tap, 3)
                            for r0, nrows in grp:
                                nc.tensor.matmul(
                                    pss[(r0, nrows)][:, : nrows * W],
                                    lhsT=lhsT(h, tap),
                                    rhs=rhs(r0, nrows, dy, dx),
                                    start=(tap == 0),
                                    stop=(tap == 8),
                                )
                        # copy each finished piece out of PSUM (fp32 -> fp16)
                        # and stream it to DRAM immediately, alternating DMA
                        # rings
                        for i, (r0, nrows) in enumerate(grp):
                            pix0, npx = r0 * W, nrows * W
                            nc.vector.tensor_copy(
                                out=ot[:, h * PIX + pix0 : h * PIX + pix0 + npx],
                                in_=pss[(r0, nrows)][:, :npx],
                            )
                            ring = nc.scalar if i % 2 == 0 else nc.sync
                            ring.dma_start(
                                out=out[
                                    n, h * 128 : (h + 1) * 128, pix0 : pix0 + npx
                                ],
                                in_=ot[:, h * PIX + pix0 : h * PIX + pix0 + npx],
                            )

    _split_sync_waits(nc, mybir)
    return nc


def _prep_inputs(input_batch, weights):
    xp = np.zeros((N_FULL, CIN, HP, WP), dtype=np.float16)
    xp[:, :, 1:-1, 1:-1] = input_batch
    xp = xp.reshape(N_FULL, CIN, PPIX)
    # w[ci, h*1152 + tap*128 + c] = weights[h*128 + c, ci, dy, dx]
    wt = np.ascontiguousarray(
        weights.astype(np.float32)
        .transpose(1, 2, 3, 0)  # [ci, dy, dx, co]
        .reshape(CIN, 3, 3, 2, 128)  # co -> (h, c)
        .transpose(0, 3, 1, 2, 4)  # [ci, h, dy, dx, c]
        .reshape(CIN, 9 * COUT)
        .astype(np.float16)
    )
    in_maps = []
    for i in range(N_CORES):
        in_maps.append(
            {
                "x": np.ascontiguousarray(xp[i * IMGS : (i + 1) * IMGS]),
                "w": wt,
            }
        )
    return in_maps


def _run(input_batch, weights, trace=False):
    from concourse.bass_utils import run_bass_kernel_spmd

    if "nc" not in _CACHE:
        _CACHE["nc"] = _build()
    nc = _CACHE["nc"]
    in_maps = _prep_inputs(np.asarray(input_batch), np.asarray(weights))
    res = run_bass_kernel_spmd(nc, in_maps, list(range(N_CORES)), trace=trace)
    outs = [
        res.results[i]["out"].reshape(IMGS, COUT, H, W).astype(np.float32)
        for i in range(N_CORES)
    ]
    full = np.concatenate(outs, axis=0)
    return full, res


def kernel(input_batch, weights):
    full, _ = _run(input_batch, weights, trace=False)
    return full


# revision 5
# speedup vs baseline: 1.0278x; 1.0278x over previous
"""Conv2D 3x3 (NCHW, OIHW, stride 1, pad 1) on 8 Trainium2 NeuronCores.

Problem shape: input (32, 128, 56, 56) fp32, weights (256, 128, 3, 3) fp32,
output (32, 256, 56, 56) fp32.

Strategy:
  - Data-parallel over batch: 4 images per core, weights replicated.
  - Host zero-pads images to 58x58 and re-lays weights as [ci, tap, co]
    so the device kernel is pure shifted matmuls.
  - Per image: for each 8-row output piece (8x56 = 448 pixels) and each
    co-half (128 of 256), accumulate 9 tap matmuls in PSUM:
        psum[co, pix] += W[tap][ci, co].T @ x_pad[ci, shifted pixels]
    contract dim = 128 channels (full partitions), moving free dim = 448.
  - fp16 operands and fp16 output DMA (upcast to fp32 on host); error
    budget is 2e-2 and this lands ~4e-4.
  - Dependencies are tile-granular, so the critical first-matmul deps live
    in their own small tiles: weights taps 0-2 of co-half 0 (wa) and input
    rows 0-9 of image 0 (xh). Image 0's remaining low rows ride the vector
    HWDGE ring in parallel with the sync ring. A short warmup burst keeps
    the PE busy (p-state ramp) while those first DMAs land.
"""

import sys

sys.path.insert(0, "/opt/trn_rl_repo")

import numpy as np

N_CORES = 8
N_FULL = 32
IMGS = N_FULL // N_CORES  # images per core
CIN = 128
COUT = 256
H = W = 56
HP = WP = 58  # padded
PIX = H * W  # 3136
PPIX = HP * WP  # 3364

_CACHE = {}

WARMUP_MM = 16


def _split_sync_waits(nc, mybir, max_waits=1):
    """The walrus build in this container rejects instructions carrying
    more than one semaphore wait; hoist extras onto preceding NOPs on the
    same engine (engine executes them in order, semantics preserved)."""
    ctr = 0
    for f in nc.m.functions:
        for bb in f.blocks:
            new_insts = []
            for ins in bb.instructions:
                si = getattr(ins, "sync_info", None)
                if si is not None and si.on_wait and len(si.on_wait) > max_waits:
                    waits = list(si.on_wait)
                    extra, keep = waits[:-max_waits], waits[-max_waits:]
                    for i in range(0, len(extra), max_waits):
                        ctr += 1
                        nop = mybir.InstNoOp(
                            name=f"{ins.name}_wsplit{ctr}",
                            engine=ins.engine,
                            sync_info=mybir.SyncInfo(
                                on_wait=extra[i : i + max_waits], on_update=[]
                            ),
                            bass_nofuse=True,
                        )
                        new_insts.append(nop)
                    si.on_wait = keep
                new_insts.append(ins)
            bb.instructions[:] = new_insts
    return ctr


# input row tiles (padded-row ranges):
#   xh: rows 0..9    (first piece of each image starts as soon as this lands)
#   xa: rows 8..33   (pieces r0=8,16,24)
#   xb: rows 32..57  (pieces r0=32,40,48)
XH_R0, XH_NR = 0, 10
XA_R0, XA_NR = 8, 26
XB_R0, XB_NR = 32, 26


def _groups(n, h):
    """Row-piece groups for image n, co-half h. Each piece is (r0, nrows);
    output pixels r0*56 .. (r0+nrows)*56, PSUM-accumulated over 9 taps."""
    if n == 0 and h == 0:
        # lead with a single piece gated only on the xh + wa tiles
        return [[(0, 8)], [(8, 8), (16, 8), (24, 8)], [(32, 8), (40, 8), (48, 8)]]
    if n == IMGS - 1 and h == 1:
        # trail with a single piece so the final exposed drain + DMA is one
        # transfer
        return [[(0, 8), (8, 8), (16, 8), (24, 8)], [(32, 8), (40, 8)], [(48, 8)]]
    return [[(0, 8), (8, 8), (16, 8), (24, 8)], [(32, 8), (40, 8), (48, 8)]]


def _build():
    import concourse.bass as bass
    import concourse.mybir as mybir
    import concourse.tile as tile

    f32 = mybir.dt.float32
    f16 = mybir.dt.float16

    nc = bass.Bass()
    x = nc.declare_dram_parameter("x", [IMGS, CIN, PPIX], f16, isOutput=False)
    w = nc.declare_dram_parameter("w", [CIN, 9 * COUT], f16, isOutput=False)
    out = nc.declare_dram_parameter("out", [IMGS, COUT, PIX], f16, isOutput=True)

    x4 = x.rearrange("n p (r c) -> n p r c", c=WP)

    with tile.TileContext(nc) as tc:
        with (
            tc.tile_pool(name="wpool", bufs=1) as wpool,
            tc.tile_pool(name="xhpool", bufs=2) as xhpool,
            tc.tile_pool(name="xapool", bufs=2) as xapool,
            tc.tile_pool(name="xbpool", bufs=2) as xbpool,
            tc.tile_pool(name="opool", bufs=2) as opool,
            tc.tile_pool(name="psum", bufs=8, space="PSUM") as pspool,
        ):
            # PE warmup while the first DMAs are in flight (p-state ramp)
            warm = wpool.tile([128, 256], f16, name="warm")
            nc.vector.memzero(warm[:])
            wps = pspool.tile([128, 256], f32, name="ps")
            for _ in range(WARMUP_MM):
                nc.tensor.matmul(
                    wps[:], lhsT=warm[:, 0:128], rhs=warm[:], start=True, stop=True
                )

            # weights on the scalar HWDGE ring, three separate tiles so the
            # first real matmul's LDWEIGHTS waits only on wa (taps 0-2, h0)
            wa = wpool.tile([CIN, 384], f16)
            wb = wpool.tile([CIN, 768], f16)
            wc = wpool.tile([CIN, 1152], f16)
            nc.scalar.dma_start(out=wa[:], in_=w[:, 0:384])
            nc.scalar.dma_start(out=wb[:], in_=w[:, 384:1152])
            nc.scalar.dma_start(out=wc[:], in_=w[:, 1152:2304])

            def lhsT(h, tap):
                if h == 1:
                    return wc[:, tap * 128 : (tap + 1) * 128]
                if tap < 3:
                    return wa[:, tap * 128 : (tap + 1) * 128]
                return wb[:, (tap - 3) * 128 : (tap - 2) * 128]

            for n in range(IMGS):
                # image 0: rows 0-9 in their own tile (xh) so the first
                # piece's matmuls wait only on that DMA + wa; its bulk low
                # rows (8-33) land separately. Other images: one low tile.
                a_r0, a_nr = (XA_R0, XA_NR) if n == 0 else (0, 34)
                xa = xapool.tile([CIN, a_nr * WP], f16)
                xb = xbpool.tile([CIN, XB_NR * WP], f16)
                xa3 = xa.rearrange("p (r c) -> p r c", c=WP)
                xb3 = xb.rearrange("p (r c) -> p r c", c=WP)
                if n == 0:
                    xh = xhpool.tile([CIN, XH_NR * WP], f16)
                    xh3 = xh.rearrange("p (r c) -> p r c", c=WP)
                    nc.sync.dma_start(out=xh[:], in_=x4[n, :, 0:XH_NR, :])
                nc.sync.dma_start(out=xa[:], in_=x4[n, :, a_r0 : a_r0 + a_nr, :])
                nc.sync.dma_start(out=xb[:], in_=x4[n, :, XB_R0 : XB_R0 + XB_NR, :])

                def rhs(r0, nrows, dy, dx):
                    if n == 0 and r0 == 0:
                        return xh3[:, dy : dy + nrows, dx : dx + W]
                    if r0 + nrows <= 32:
                        ra = r0 + dy - a_r0
                        return xa3[:, ra : ra + nrows, dx : dx + W]
                    rb = r0 + dy - XB_R0
                    return xb3[:, rb : rb + nrows, dx : dx + W]

                ot = opool.tile([128, 2 * PIX], f16)
                for h in range(2):
                    for grp in _groups(n, h):
                        pss = {}
                        for piece in grp:
                            pss[piece] = pspool.tile([128, 448], f32, name="ps")
                        for tap in range(9):
                            dy, dx = divmod(tap, 3)
                            for r0, nrows in grp:
                                nc.tensor.matmul(
                                    pss[(r0, nrows)][:, : nrows * W],
                                    lhsT=lhsT(h, tap),
                                    rhs=rhs(r0, nrows, dy, dx),
                                    start=(tap == 0),
                                    stop=(tap == 8),
                                )
                        # copy each finished piece out of PSUM (fp32 -> fp16)
                        # and stream it to DRAM immediately, alternating DMA
                        # rings
                        for i, (r0, nrows) in enumerate(grp):
                            pix0, npx = r0 * W, nrows * W
                            nc.vector.tensor_copy(
                                out=ot[:, h * PIX + pix0 : h * PIX + pix0 + npx],
                                in_=pss[(r0, nrows)][:, :npx],
                            )
                            ring = nc.scalar if i % 2 == 0 else nc.sync
                            ring.dma_start(
                                out=out[
                                    n, h * 128 : (h + 1) * 128, pix0 : pix0 + npx
                                ],
                                in_=ot[:, h * PIX + pix0 : h * PIX + pix0 + npx],
                            )

    _split_sync_waits(nc, mybir)
    return nc


def _prep_inputs(input_batch, weights):
    xp = np.zeros((N_FULL, CIN, HP, WP), dtype=np.float16)
    xp[:, :, 1:-1, 1:-1] = input_batch
    xp = xp.reshape(N_FULL, CIN, PPIX)
    # w[ci, h*1152 + tap*128 + c] = weights[h*128 + c, ci, dy, dx]
    wt = np.ascontiguousarray(
        weights.astype(np.float32)
        .transpose(1, 2, 3, 0)  # [ci, dy, dx, co]
        .reshape(CIN, 3, 3, 2, 128)  # co -> (h, c)
        .transpose(0, 3, 1, 2, 4)  # [ci, h, dy, dx, c]
        .reshape(CIN, 9 * COUT)
        .astype(np.float16)
    )
    in_maps = []
    for i in range(N_CORES):
        in_maps.append(
            {
                "x": np.ascontiguousarray(xp[i * IMGS : (i + 1) * IMGS]),
                "w": wt,
            }
        )
    return in_maps


def _run(input_batch, weights, trace=False):
    from concourse.bass_utils import run_bass_kernel_spmd

    if "nc" not in _CACHE:
        _CACHE["nc"] = _build()
    nc = _CACHE["nc"]
    in_maps = _prep_inputs(np.asarray(input_batch), np.asarray(weights))
    res = run_bass_kernel_spmd(nc, in_maps, list(range(N_CORES)), trace=trace)
    outs = [
        res.results[i]["out"].reshape(IMGS, COUT, H, W).astype(np.float32)
        for i in range(N_CORES)
    ]
    full = np.concatenate(outs, axis=0)
    return full, res


def kernel(input_batch, weights):
    full, _ = _run(input_batch, weights, trace=False)
    return full
